# revision 6
# baseline (speedup 1.0000x reference)
"""Single-head causal attention (B=8, T=2048, C=1024, H=128) on 8 TRN2 NeuronCores.

Sharding: data-parallel over batch B — core b computes attention for x[b].
Host-side prep per core: x[b] is transposed to xT [C, T] (so the contraction
dim C lands on SBUF partitions) and the softmax scale C**-0.5 is folded into
Wq. Device kernel per core:
  1. QT/KT/VT = W.T @ X.T -> [H, T] layouts (PE, contraction over C, PSUM acc)
  2. V [T, H] tiles from VT via one DMA-xbar transpose
  3. per 128-row q-tile i: S = QT_i.T @ KT (causal width) on PE, diag masked
     (DVE add on PSUM), exp on ACT (accum_out -> row sums), ONE DMA-xbar
     transpose exp(S) -> [s, q] tiles, AV = expST.T @ V accumulated in PSUM,
     normalize by 1/rowsum (DVE), DMA out.
"""

import os
from contextlib import ExitStack

import numpy as np
import ml_dtypes

B, T, C, H = 8, 2048, 1024, 128
P = 128
NT = T // P  # 16 q/s tiles per core
NCC = C // P  # 8 contraction chunks
N_CORES = 8
SCALE = float(C) ** -0.5

# "bf16": x/W/Q/K in bf16 (full-rate matmuls, FWL weight loads)
# "f32r": x/W/Q/K stored fp32, matmuls in relaxed-fp32 mode
DT_CONFIG = os.environ.get("ATTN_DT", "bf16")

_CACHE = {}


def _build(dt_config):
    import concourse.bass as bass
    import concourse.mybir as mybir
    import concourse.tile as tile
    from concourse import bacc
    from concourse.masks import make_causal_mask

    dt = mybir.dt
    dt_in = dt.bfloat16 if dt_config == "bf16" else dt.float32r
    dt_av = dt.bfloat16  # exp(S) / V dtype feeding the AV matmuls (xbar needs 2B)
    f32 = dt.float32

    nc = bacc.Bacc("TRN2", target_bir_lowering=False, debug=False)
    xT = nc.dram_tensor("xT", [C, T], dt_in, kind="ExternalInput").ap()
    wq = nc.dram_tensor("wq", [C, H], dt_in, kind="ExternalInput").ap()
    wk = nc.dram_tensor("wk", [C, H], dt_in, kind="ExternalInput").ap()
    wv = nc.dram_tensor("wv", [C, H], dt_in, kind="ExternalInput").ap()
    out = nc.dram_tensor("out", [T, H], f32, kind="ExternalOutput").ap()

    CH = 1024  # scores PSUM chunk width (2 banks)

    with tile.TileContext(nc) as tc, ExitStack() as ctx:
        consts = ctx.enter_context(tc.tile_pool(name="consts", bufs=1))
        cmask = consts.tile([P, P], f32)
        make_causal_mask(nc, cmask, mask_val=-30000.0)
        # prime DVE's vector clock against the gpsimd-built constant, so the
        # first real DVE consumer doesn't need an extra sync-wait slot
        prime = consts.tile([P, P], f32)
        nc.vector.tensor_copy(prime, cmask)

        wpool = ctx.enter_context(tc.tile_pool(name="wpool", bufs=1))
        w_sb = {}
        for name, dram in (("wq", wq), ("wk", wk), ("wv", wv)):
            wt = wpool.tile([P, NCC * H], dt_in, name=f"{name}_sb")
            nc.sync.dma_start(
                wt.rearrange("p (c h) -> p c h", c=NCC),
                dram.rearrange("(c p) h -> p c h", p=P),
            )
            w_sb[name] = wt

        # x load split into (n2, c) pieces, n2-major, so the first projection
        # pass (n2=0) can start after only 1/2 of the input has landed
        xpool = ctx.enter_context(tc.tile_pool(name="xpool", bufs=1))
        xt_sb = xpool.tile([P, NCC * T], dt_in)
        for n2 in range(2):
            for c in range(NCC):
                nc.sync.dma_start(
                    xt_sb[:, c * T + n2 * CH : c * T + (n2 + 1) * CH],
                    xT[c * P : (c + 1) * P, n2 * CH : (n2 + 1) * CH],
                )

        qkv = ctx.enter_context(tc.tile_pool(name="qkv", bufs=1))
        qt_sb = qkv.tile([P, T], dt_in)
        kt_sb = qkv.tile([P, T], dt_in)
        vt_sb = qkv.tile([P, T], dt_av)

        # PSUM: ps_mm [128,1024] x3 = 6 banks (proj + scores), ps_av 2 banks
        ps_mm = ctx.enter_context(tc.tile_pool(name="ps_mm", bufs=3, space="PSUM"))
        ps_av = ctx.enter_context(tc.tile_pool(name="ps_av", bufs=2, space="PSUM"))

        # --- projections: QT/KT/VT [H, T] accumulated over C chunks ---
        for n2 in range(2):
            for pname, dst in (("wq", qt_sb), ("wk", kt_sb), ("wv", vt_sb)):
                wt = w_sb[pname]
                ps = ps_mm.tile([P, CH], f32, name=f"psp_{pname}_{n2}", tag="psmm")
                for half in range(2):
                    n = n2 * 2 + half
                    for c in range(NCC):
                        nc.tensor.matmul(
                            ps[:, half * 512 : (half + 1) * 512],
                            wt[:, c * H : (c + 1) * H],
                            xt_sb[:, c * T + n * 512 : c * T + (n + 1) * 512],
                            start=(c == 0),
                            stop=(c == NCC - 1),
                        )
                nc.scalar.copy(dst[:, n2 * CH : (n2 + 1) * CH], ps)

        # --- V natural layout [T, H] via one DMA-xbar transpose ---
        vpool = ctx.enter_context(tc.tile_pool(name="vpool", bufs=1))
        v_sb = vpool.tile([P, NT * H], dt_av)
        nc.sync.dma_start(
            v_sb.rearrange("p (t h) -> p t h", t=NT), vt_sb, transpose=True
        )

        # --- attention ---
        exps_pool = ctx.enter_context(tc.tile_pool(name="exps_pool", bufs=2))
        expst_pool = ctx.enter_context(tc.tile_pool(name="expst_pool", bufs=2))
        sums_pool = ctx.enter_context(tc.tile_pool(name="sums_pool", bufs=2))
        outp = ctx.enter_context(tc.tile_pool(name="outp", bufs=2))

        exps_tiles = [None] * NT
        sums_tiles = [None] * NT
        expst_tiles = [None] * NT

        def emit_scores(i):
            w_s = (i + 1) * P  # causal width of this q-tile's score row
            nch = (w_s + CH - 1) // CH
            exps = exps_pool.tile([P, w_s], dt_av, name=f"exps{i}", tag="exps")
            sums = sums_pool.tile([P, nch], f32, name=f"sums{i}", tag="sums")
            for ci in range(nch):
                base = ci * CH
                cw = min(CH, w_s - base)
                ps = ps_mm.tile([P, CH], f32, name=f"ps_s{i}_{ci}", tag="psmm")
                for s0 in range(0, cw, 512):
                    sw = min(512, cw - s0)
                    nc.tensor.matmul(
                        ps[:, s0 : s0 + sw],
                        qt_sb[:, i * P : (i + 1) * P],
                        kt_sb[:, base + s0 : base + s0 + sw],
                        start=True,
                        stop=True,
                    )
                if base <= i * P < base + cw:  # diagonal tile lives here
                    off = i * P - base
                    nc.vector.tensor_add(
                        ps[:, off : off + P], ps[:, off : off + P], cmask
                    )
                nc.scalar.activation(
                    exps[:, base : base + cw],
                    ps[:, :cw],
                    mybir.ActivationFunctionType.Exp,
                    accum_out=sums[:, ci : ci + 1],
                )
            exps_tiles[i] = exps
            sums_tiles[i] = sums

        def emit_transpose(i):
            exps = exps_tiles[i]
            expst = expst_pool.tile(
                [P, (i + 1) * P], dt_av, name=f"expst{i}", tag="expst"
            )
            nc.sync.dma_start(
                expst.rearrange("p (t q) -> p t q", t=i + 1),
                exps,
                transpose=True,
            )
            expst_tiles[i] = expst

        def emit_av(i):
            sums = sums_tiles[i]
            nch = ((i + 1) * P + CH - 1) // CH
            expst = expst_tiles[i]
            pav = ps_av.tile([P, H], f32, name=f"pav{i}", tag="ps_av")
            for j in range(i + 1):
                nc.tensor.matmul(
                    pav,
                    expst[:, j * P : (j + 1) * P],
                    v_sb[:, j * H : (j + 1) * H],
                    start=(j == 0),
                    stop=(j == i),
                )
            rec = sums_pool.tile([P, 1], f32, name=f"rec{i}", tag="rec")
            if nch == 1:
                nc.vector.reciprocal(rec, sums[:, 0:1])
            else:
                tot = sums_pool.tile([P, 1], f32, name=f"tot{i}", tag="tot")
                nc.vector.reduce_sum(tot, sums, axis=mybir.AxisListType.X)
                nc.vector.reciprocal(rec, tot)
            o = outp.tile([P, H], f32, name=f"o{i}", tag="o")
            nc.vector.tensor_scalar_mul(o, pav, rec)
            nc.sync.dma_start(out[i * P : (i + 1) * P, :], o)

        # software pipeline: scores i+1 and the xbar transpose of i+1 land
        # before AV of i, so PE / ACT / DMA overlap across q-tiles.
        emit_scores(0)
        emit_transpose(0)
        for i in range(NT):
            if i + 1 < NT:
                emit_scores(i + 1)
                emit_transpose(i + 1)
            emit_av(i)

    nc.compile()
    return nc


def _get_bass():
    if DT_CONFIG not in _CACHE:
        _CACHE[DT_CONFIG] = _build(DT_CONFIG)
    return _CACHE[DT_CONFIG]


LAST_RESULT = None  # BassKernelResults of the most recent kernel() call


def kernel(x, Wq, Wk, Wv):
    global LAST_RESULT
    from concourse.bass_utils import run_bass_kernel_spmd

    np_dt = ml_dtypes.bfloat16 if DT_CONFIG == "bf16" else np.float32
    wq_s = (np.asarray(Wq, np.float32) * SCALE).astype(np_dt)
    wk_s = np.asarray(Wk, np.float32).astype(np_dt)
    wv_s = np.asarray(Wv, np.float32).astype(np_dt)
    x = np.asarray(x, np.float32)

    in_maps = []
    for b in range(N_CORES):
        in_maps.append(
            {
                "xT": np.ascontiguousarray(x[b].T).astype(np_dt),
                "wq": wq_s,
                "wk": wk_s,
                "wv": wv_s,
            }
        )

    nc = _get_bass()
    res = run_bass_kernel_spmd(nc, in_maps, core_ids=list(range(N_CORES)))
    LAST_RESULT = res
    return np.stack([r["out"] for r in res.results], axis=0)
